# revision 16
# baseline (speedup 1.0000x reference)
"""Multi-head attention (B=2, S=2048, D=1024, H=16) on 8 TRN2 NeuronCores.

Sharding: data-parallel over batch (2 groups) x tensor-parallel over heads
(4 head-groups of 4 heads = 256 features). Each core computes Q/K/V
projections for its head slice, attention for its 4 heads, and a partial
output projection (Wo column slice). Partials are summed on the host
(gather/unshard step) - no on-device collective.

Device layout (everything transposed, [feature, seq], so biases are
per-partition scalars and no on-chip transposes are needed):
  qTd/kTd[h] [128, S]   per-head Q^T/K^T with the 64-dim head feature
                        DUPLICATED into rows 64..127. The QK^T matmul then
                        contracts over 128 partitions (full PE row
                        utilization; a 64-partition moving operand runs ~2x
                        slower under engine concurrency) and computes
                        2*(q.k), absorbed into the exp scale (1/16).
  scoresT   = kTd_chunk.T @ qTd  -> [128 keys, 512 q] PSUM
  attnT     = exp(scoresT/16)    -> SBUF (ACT; the critical rate:
                                   ~578ns/tile, the attention floor)
  ctxT      = [V_h | 1].T @ attnT accumulated on 2 ALTERNATING PSUM banks
              (same-bank back-to-back accumulation serializes ~2.6x) ->
              rows 0..63 ctx, row 64 softmax denominator
  yT       += WoT_g.T @ ctxT     partial, summed on host
"""

import os
import sys

for _p in (
    "/root/.axon_site",
    "/root/.axon_site/_ro/trn_rl_repo",
    "/root/.axon_site/_ro/pypackages",
    "/opt/trn_rl_repo",
):
    if os.path.isdir(_p) and _p not in sys.path:
        sys.path.append(_p)

import numpy as np

import concourse.bass as bass
import concourse.mybir as mybir
from concourse import bacc
from concourse.tile import TileContext
from concourse.bass_utils import run_bass_kernel_spmd

B, S, D, H = 2, 2048, 1024, 16
DK = D // H  # 64
N_CORES, DP, TP = 8, 2, 4
HPC = H // TP  # 4 heads per core
HF = HPC * DK  # 256 head-features per core

F32 = mybir.dt.float32
MM_DT = {
    "fp32r": mybir.dt.float32r,
    "bf16": mybir.dt.bfloat16,
}[os.environ.get("MM_DT", "fp32r")]
MM_NP = mybir.dt.np(MM_DT)

QT_TILES = HF // 128  # 2
KCH = D // 128  # 8 contraction chunks for projections
ST512 = S // 512  # 4 seq tiles of 512
ST128 = S // 128  # 16 seq tiles of 128
VW = DK + 1  # 65: per-head V columns + ones column
VROW = HPC * VW  # 260
EXPF = mybir.ActivationFunctionType.Exp

LAST_RESULTS = None  # test harness reads exec_time_ns from here


def _build_nc():
    nc = bacc.Bacc("TRN2", target_bir_lowering=False, debug=False,
                   num_devices=N_CORES)

    xq = nc.dram_tensor("xqT", [D, S], MM_DT, kind="ExternalInput")
    xk = nc.dram_tensor("xkT", [D, S], MM_DT, kind="ExternalInput")
    xv = nc.dram_tensor("xvT", [D, S], MM_DT, kind="ExternalInput")
    wq = nc.dram_tensor("wqT", [D, HF], MM_DT, kind="ExternalInput")
    wk = nc.dram_tensor("wkT", [D, HF], MM_DT, kind="ExternalInput")
    bq = nc.dram_tensor("bq", [HF, 1], F32, kind="ExternalInput")
    bk = nc.dram_tensor("bk", [HF, 1], F32, kind="ExternalInput")
    wv = nc.dram_tensor("wv_ext", [D + 1, VROW], MM_DT, kind="ExternalInput")
    wo = nc.dram_tensor("woT", [HF, D], MM_DT, kind="ExternalInput")
    ones_d = nc.dram_tensor("ones1", [1, 128], MM_DT, kind="ExternalInput")
    yt = nc.dram_tensor("yT", [D, S], F32, kind="ExternalOutput")

    from contextlib import ExitStack

    with TileContext(nc) as tc, ExitStack() as ctx:
        persist = ctx.enter_context(tc.tile_pool(name="persist", bufs=1))
        ps_big = ctx.enter_context(tc.tile_pool(name="ps_big", bufs=4, space="PSUM"))
        ps_acc = ctx.enter_context(tc.tile_pool(name="ps_acc", bufs=1, space="PSUM"))
        ps_qp = ctx.enter_context(tc.tile_pool(name="ps_qp", bufs=2, space="PSUM"))
        qproj = ctx.enter_context(tc.tile_pool(name="qproj", bufs=1))
        xload = ctx.enter_context(tc.tile_pool(name="xload", bufs=2))
        proj_ctx = ExitStack()
        projw = proj_ctx.enter_context(tc.tile_pool(name="projw", bufs=1))
        vload = proj_ctx.enter_context(tc.tile_pool(name="vload", bufs=2))

        # --- weights (k/v proj weights live only for the proj phase) ---
        wq_sb = qproj.tile([128, KCH, HF], MM_DT, name="wq_sb", tag="wq_sb")
        wk_sb = projw.tile([128, KCH, HF], MM_DT, name="wk_sb", tag="wk_sb")
        wv_sb = projw.tile([128, KCH, VROW], MM_DT, name="wv_sb", tag="wv_sb")
        wv_row = projw.tile([1, VROW], MM_DT, name="wv_row", tag="wv_row")
        wo_sb = [persist.tile([128, D], MM_DT, name=f"wo{t}", tag=f"wo{t}") for t in range(QT_TILES)]
        bq_sb = [persist.tile([128, 1], F32, name=f"bq{t}", tag=f"bq{t}") for t in range(QT_TILES)]
        bk_sb = [persist.tile([128, 1], F32, name=f"bk{t}", tag=f"bk{t}") for t in range(QT_TILES)]
        ones1 = projw.tile([1, 128], MM_DT, name="ones1", tag="ones1")
        nc.sync.dma_start(out=ones1, in_=ones_d[:, :])
        nc.sync.dma_start(out=wq_sb, in_=wq.ap().rearrange("(c p) f -> p c f", p=128))
        nc.sync.dma_start(out=wk_sb, in_=wk.ap().rearrange("(c p) f -> p c f", p=128))
        nc.sync.dma_start(out=wv_sb, in_=wv.ap()[:D, :].rearrange("(c p) f -> p c f", p=128))
        nc.sync.dma_start(out=wv_row, in_=wv[D:D + 1, :])
        for t in range(QT_TILES):
            nc.sync.dma_start(out=wo_sb[t], in_=wo[t * 128:(t + 1) * 128, :])
            nc.sync.dma_start(out=bq_sb[t], in_=bq[t * 128:(t + 1) * 128, :])
            nc.sync.dma_start(out=bk_sb[t], in_=bk[t * 128:(t + 1) * 128, :])

        # --- persistent activations: per-head dup-dk Q^T/K^T, V ---
        qTd = [persist.tile([128, S], MM_DT, name=f"qTd{h}", tag=f"qTd{h}") for h in range(HPC)]
        kTd = [persist.tile([128, S], MM_DT, name=f"kTd{h}", tag=f"kTd{h}") for h in range(HPC)]
        v_sb = [persist.tile([128, VROW], MM_DT, name=f"v{s}", tag=f"v{s}") for s in range(ST128)]

        # --- K/Q projections into duplicated-dk per-head layout ---
        def proj_dup(xdram, w_chunks, b_chunks, dst, name, sts=range(ST512),
                     pspool=None):
            for st in sts:
                xt = xload.tile([128, KCH, 512], MM_DT, name=f"x{name}", tag="xqk")
                xr = xdram.ap().rearrange("(c p) s -> p c s", p=128)
                half = KCH // 2
                nc.sync.dma_start(
                    out=xt[:, :half, :],
                    in_=xr[:, :half, st * 512:(st + 1) * 512])
                nc.sync.dma_start(
                    out=xt[:, half:, :],
                    in_=xr[:, half:, st * 512:(st + 1) * 512])
                for t in range(QT_TILES):
                    pool = pspool or ps_big
                    ps = pool.tile([128, 512], F32, name="psqk",
                                   tag="psqp" if pspool else "psqk")
                    for c in range(KCH):
                        nc.tensor.matmul(
                            ps, w_chunks[:, c, t * 128:(t + 1) * 128],
                            xt[:, c, :], start=(c == 0), stop=(c == KCH - 1))
                    sl = slice(st * 512, (st + 1) * 512)
                    for half in range(2):
                        h = 2 * t + half
                        rows = slice(half * 64, half * 64 + 64)
                        nc.vector.tensor_scalar_add(
                            dst[h][0:64, sl], ps[rows, :], b_chunks[t][rows, :])
                        nc.vector.tensor_scalar_add(
                            dst[h][64:128, sl], ps[rows, :], b_chunks[t][rows, :])

        proj_dup(xk, wk_sb, bk_sb, kTd, "k")

        # --- V projection (fused bias + ones column via extra K row) ---
        for s in range(ST128):
            vt = vload.tile([128, KCH, 128], MM_DT, name="xv", tag="xv")
            nc.sync.dma_start(
                out=vt,
                in_=xv.ap().rearrange("(c p) s -> p c s", p=128)[
                    :, :, s * 128:(s + 1) * 128])
            ps = ps_big.tile([128, 512], F32, name="psv", tag="psqk")
            for c in range(KCH):
                nc.tensor.matmul(ps[:, :VROW], vt[:, c, :], wv_sb[:, c, :],
                                 start=(c == 0), stop=False)
            nc.tensor.matmul(ps[:, :VROW], ones1, wv_row,
                             start=False, stop=True)
            nc.vector.tensor_copy(v_sb[s], ps[:, :VROW])

        proj_ctx.close()
        attn_sb = ctx.enter_context(tc.tile_pool(name="attn_sb", bufs=8))
        stage = ctx.enter_context(tc.tile_pool(name="stage", bufs=2))
        misc = ctx.enter_context(tc.tile_pool(name="misc", bufs=1))
        outcp = ctx.enter_context(tc.tile_pool(name="outcp", bufs=3))

        # --- attention + output projection, qt-outer; Q proj streams in ---
        inv_scale = 1.0 / (2.0 * np.sqrt(DK))  # dup-dk doubles the dot
        proj_dup(xq, wq_sb, bq_sb, qTd, "q", sts=[0], pspool=ps_qp)
        for qt in range(ST512):
            qsl = slice(qt * 512, (qt + 1) * 512)
            ctx_stage = [stage.tile([VW, 512], F32, name=f"cs{h}", tag=f"cs{h}")
                         for h in range(HPC)]
            dn4 = misc.tile([128, 512], F32, name="dn4", tag="dn4")
            for h in range(HPC):
                if h == 2 and qt + 1 < ST512:
                    proj_dup(xq, wq_sb, bq_sb, qTd, "q", sts=[qt + 1],
                             pspool=ps_qp)
                pcs = [ps_acc.tile([VW, 512], F32, name=f"pc{j}", tag=f"pc{j}")
                       for j in range(2)]
                for kc in range(ST128):
                    ps = ps_big.tile([128, 512], F32, name="pss", tag="psqk")
                    nc.tensor.matmul(ps, kTd[h][:, kc * 128:(kc + 1) * 128],
                                     qTd[h][:, qsl], start=True, stop=True)
                    at = attn_sb.tile([128, 512], MM_DT, name="at", tag="at")
                    nc.scalar.activation(out=at, in_=ps, func=EXPF,
                                         scale=inv_scale)
                    nc.tensor.matmul(pcs[kc % 2],
                                     v_sb[kc][:, h * VW:(h + 1) * VW], at,
                                     start=(kc < 2), stop=(kc >= ST128 - 2))
                # combine the two banks; row 64 is the denominator
                tmp = misc.tile([VW, 512], F32, name="tmp", tag="tmp", bufs=2)
                nc.vector.tensor_copy(tmp, pcs[0])
                nc.vector.tensor_add(ctx_stage[h], pcs[1], tmp)
                nc.vector.tensor_copy(dn4[32 * h:32 * h + 1, :],
                                      ctx_stage[h][64:65, :])
            rec4 = misc.tile([128, 512], F32, name="rec4", tag="rec4")
            rscr = misc.tile([128, 512], F32, name="rscr", tag="rscr")
            nc.vector.reciprocal_approx_accurate(rec4[0:97, :], dn4[0:97, :],
                                                 rscr[0:97, :])
            cT = [stage.tile([128, 512], MM_DT, name=f"cT{t}", tag=f"cT{t}")
                  for t in range(QT_TILES)]
            for h in range(HPC):
                rec1 = misc.tile([1, 512], F32, name="rec1", tag="rec1")
                nc.vector.tensor_copy(rec1, rec4[32 * h:32 * h + 1, :])
                rb = misc.tile([64, 512], F32, name="rb", tag="rb")
                nc.gpsimd.partition_broadcast(rb, rec1)
                nc.vector.tensor_mul(
                    cT[h // 2][(h % 2) * 64:(h % 2) * 64 + 64, :],
                    ctx_stage[h][0:64, :], rb)
            # partial output projection for this qt
            for ot in range(D // 128):
                po = ps_big.tile([128, 512], F32, name="pso", tag="psqk")
                for t in range(QT_TILES):
                    nc.tensor.matmul(po, wo_sb[t][:, ot * 128:(ot + 1) * 128],
                                     cT[t], start=(t == 0),
                                     stop=(t == QT_TILES - 1))
                yo = outcp.tile([128, 512], F32, name="yo", tag="yo")
                nc.vector.tensor_copy(yo, po)
                nc.sync.dma_start(out=yt[ot * 128:(ot + 1) * 128, qsl], in_=yo)

    nc.compile()
    return nc


_NC_CACHE = {}


def kernel(q, k, v, mask, Wq, bq, Wk, bk, Wv, bv, Wo, bo):
    global LAST_RESULTS
    q = np.asarray(q, dtype=np.float32)
    k = np.asarray(k, dtype=np.float32)
    v = np.asarray(v, dtype=np.float32)

    if "nc" not in _NC_CACHE:
        _NC_CACHE["nc"] = _build_nc()
    nc = _NC_CACHE["nc"]

    def cvt(a):
        return np.ascontiguousarray(np.asarray(a, dtype=np.float32)).astype(MM_NP)

    xqT = [cvt(q[b].T) for b in range(B)]
    xkT = [cvt(k[b].T) for b in range(B)]
    xvT = [cvt(v[b].T) for b in range(B)]

    in_maps = []
    for core in range(N_CORES):
        b, g = divmod(core, TP)
        F = slice(g * HF, (g + 1) * HF)
        # wv_ext: [D+1, VROW]; per head block of 65 cols = [Wv.T | bias/1]
        wv_ext = np.zeros((D + 1, VROW), dtype=np.float32)
        for j in range(HPC):
            feat = slice(g * HF + j * DK, g * HF + (j + 1) * DK)
            wv_ext[:D, j * VW:j * VW + DK] = np.asarray(Wv)[feat, :].T
            wv_ext[D, j * VW:j * VW + DK] = np.asarray(bv)[feat]
            wv_ext[D, j * VW + DK] = 1.0
        in_maps.append({
            "xqT": xqT[b], "xkT": xkT[b], "xvT": xvT[b],
            "wqT": cvt(np.asarray(Wq)[F, :].T),
            "wkT": cvt(np.asarray(Wk)[F, :].T),
            "bq": np.ascontiguousarray(np.asarray(bq)[F].reshape(HF, 1)),
            "bk": np.ascontiguousarray(np.asarray(bk)[F].reshape(HF, 1)),
            "wv_ext": wv_ext.astype(MM_NP),
            "woT": cvt(np.asarray(Wo)[:, F].T),
            "ones1": np.ones((1, 128), dtype=MM_NP),
        })

    res = run_bass_kernel_spmd(nc, in_maps, list(range(N_CORES)))
    LAST_RESULTS = res

    out = np.zeros((B, S, D), dtype=np.float32)
    for b in range(B):
        acc = np.zeros((D, S), dtype=np.float32)
        for g in range(TP):
            acc += res.results[b * TP + g]["yT"]
        out[b] = acc.T + np.asarray(bo)[None, :]
    return out


# revision 17
# speedup vs baseline: 1.2532x; 1.2532x over previous
"""Multi-head attention (B=2, S=2048, D=1024, H=16) on 8 TRN2 NeuronCores.

Sharding: data-parallel over batch (2 groups) x tensor-parallel over heads
(4 head-groups of 4 heads = 256 features). Each core computes Q/K/V
projections for its head slice, attention for its 4 heads, and a partial
output projection (Wo column slice). Partials are summed on the host
(gather/unshard step) - no on-device collective.

Device layout (everything transposed, [feature, seq], so biases are
per-partition scalars and no on-chip transposes are needed):
  qTd/kTd[h] [128, S]   per-head Q^T/K^T with the 64-dim head feature
                        DUPLICATED into rows 64..127. The QK^T matmul then
                        contracts over 128 partitions (full PE row
                        utilization; a 64-partition moving operand runs ~2x
                        slower under engine concurrency) and computes
                        2*(q.k), absorbed into the exp scale (1/16).
  scoresT   = kTd_chunk.T @ qTd  -> [128 keys, 512 q] PSUM
  attnT     = exp(scoresT/16)    -> SBUF (ACT; the critical rate:
                                   ~578ns/tile, the attention floor)
  ctxT      = [V_h | 1].T @ attnT accumulated on 2 ALTERNATING PSUM banks
              (same-bank back-to-back accumulation serializes ~2.6x) ->
              rows 0..63 ctx, row 64 softmax denominator
  yT       += WoT_g.T @ ctxT     partial, summed on host
"""

import os
import sys

for _p in (
    "/root/.axon_site",
    "/root/.axon_site/_ro/trn_rl_repo",
    "/root/.axon_site/_ro/pypackages",
    "/opt/trn_rl_repo",
):
    if os.path.isdir(_p) and _p not in sys.path:
        sys.path.append(_p)

import numpy as np

import concourse.bass as bass
import concourse.mybir as mybir
from concourse import bacc
from concourse.tile import TileContext
from concourse.bass_utils import run_bass_kernel_spmd

B, S, D, H = 2, 2048, 1024, 16
DK = D // H  # 64
N_CORES, DP, TP = 8, 2, 4
HPC = H // TP  # 4 heads per core
HF = HPC * DK  # 256 head-features per core

F32 = mybir.dt.float32
MM_DT = {
    "fp32r": mybir.dt.float32r,
    "bf16": mybir.dt.bfloat16,
}[os.environ.get("MM_DT", "fp32r")]
MM_NP = mybir.dt.np(MM_DT)

QT_TILES = HF // 128  # 2
KCH = D // 128  # 8 contraction chunks for projections
ST512 = S // 512  # 4 seq tiles of 512
ST128 = S // 128  # 16 seq tiles of 128
VW = DK + 1  # 65: per-head V columns + ones column
VROW = HPC * VW  # 260
EXPF = mybir.ActivationFunctionType.Exp

LAST_RESULTS = None  # test harness reads exec_time_ns from here


def _build_nc():
    nc = bacc.Bacc("TRN2", target_bir_lowering=False, debug=False,
                   num_devices=N_CORES)

    xq = nc.dram_tensor("xqT", [D, S], MM_DT, kind="ExternalInput")
    xk = nc.dram_tensor("xkT", [D, S], MM_DT, kind="ExternalInput")
    xv = nc.dram_tensor("xvT", [D, S], MM_DT, kind="ExternalInput")
    wq = nc.dram_tensor("wqT", [D, HF], MM_DT, kind="ExternalInput")
    wk = nc.dram_tensor("wkT", [D, HF], MM_DT, kind="ExternalInput")
    bq = nc.dram_tensor("bq", [HF, 1], F32, kind="ExternalInput")
    bk = nc.dram_tensor("bk", [HF, 1], F32, kind="ExternalInput")
    wv = nc.dram_tensor("wv_ext", [D + 1, VROW], MM_DT, kind="ExternalInput")
    wo = nc.dram_tensor("woT", [HF, D], MM_DT, kind="ExternalInput")
    ones_d = nc.dram_tensor("ones1", [1, 128], MM_DT, kind="ExternalInput")
    yt = nc.dram_tensor("yT", [D, S], F32, kind="ExternalOutput")

    from contextlib import ExitStack

    with TileContext(nc) as tc, ExitStack() as ctx:
        persist = ctx.enter_context(tc.tile_pool(name="persist", bufs=1))
        ps_big = ctx.enter_context(tc.tile_pool(name="ps_big", bufs=4, space="PSUM"))
        ps_acc = ctx.enter_context(tc.tile_pool(name="ps_acc", bufs=1, space="PSUM"))
        ps_qp = ctx.enter_context(tc.tile_pool(name="ps_qp", bufs=1, space="PSUM"))
        ps_out = ctx.enter_context(tc.tile_pool(name="ps_out", bufs=1, space="PSUM"))
        qproj = ctx.enter_context(tc.tile_pool(name="qproj", bufs=1))
        xload = ctx.enter_context(tc.tile_pool(name="xload", bufs=2))
        proj_ctx = ExitStack()
        projw = proj_ctx.enter_context(tc.tile_pool(name="projw", bufs=1))
        vload = proj_ctx.enter_context(tc.tile_pool(name="vload", bufs=2))

        # --- weights (k/v proj weights live only for the proj phase) ---
        wq_sb = qproj.tile([128, KCH, HF], MM_DT, name="wq_sb", tag="wq_sb")
        wk_sb = projw.tile([128, KCH, HF], MM_DT, name="wk_sb", tag="wk_sb")
        wv_sb = projw.tile([128, KCH, VROW], MM_DT, name="wv_sb", tag="wv_sb")
        wv_row = projw.tile([1, VROW], MM_DT, name="wv_row", tag="wv_row")
        wo_sb = [persist.tile([128, D], MM_DT, name=f"wo{t}", tag=f"wo{t}") for t in range(QT_TILES)]
        bq_sb = [persist.tile([128, 1], F32, name=f"bq{t}", tag=f"bq{t}") for t in range(QT_TILES)]
        bk_sb = [persist.tile([128, 1], F32, name=f"bk{t}", tag=f"bk{t}") for t in range(QT_TILES)]
        ones1 = projw.tile([1, 128], MM_DT, name="ones1", tag="ones1")
        nc.sync.dma_start(out=ones1, in_=ones_d[:, :])
        nc.sync.dma_start(out=wq_sb, in_=wq.ap().rearrange("(c p) f -> p c f", p=128))
        nc.sync.dma_start(out=wk_sb, in_=wk.ap().rearrange("(c p) f -> p c f", p=128))
        nc.sync.dma_start(out=wv_sb, in_=wv.ap()[:D, :].rearrange("(c p) f -> p c f", p=128))
        nc.sync.dma_start(out=wv_row, in_=wv[D:D + 1, :])
        for t in range(QT_TILES):
            nc.sync.dma_start(out=wo_sb[t], in_=wo[t * 128:(t + 1) * 128, :])
            nc.sync.dma_start(out=bq_sb[t], in_=bq[t * 128:(t + 1) * 128, :])
            nc.sync.dma_start(out=bk_sb[t], in_=bk[t * 128:(t + 1) * 128, :])

        # --- persistent activations: per-head dup-dk Q^T/K^T, V ---
        qTd = [persist.tile([128, S], MM_DT, name=f"qTd{h}", tag=f"qTd{h}") for h in range(HPC)]
        kTd = [persist.tile([128, S], MM_DT, name=f"kTd{h}", tag=f"kTd{h}") for h in range(HPC)]
        v_sb = [persist.tile([128, VROW], MM_DT, name=f"v{s}", tag=f"v{s}") for s in range(ST128)]

        # --- K/Q projections into duplicated-dk per-head layout ---
        def proj_dup(xdram, w_chunks, b_chunks, dst, name, sts=range(ST512),
                     pspool=None):
            for st in sts:
                xt = xload.tile([128, KCH, 512], MM_DT, name=f"x{name}", tag="xqk")
                xr = xdram.ap().rearrange("(c p) s -> p c s", p=128)
                half = KCH // 2
                nc.sync.dma_start(
                    out=xt[:, :half, :],
                    in_=xr[:, :half, st * 512:(st + 1) * 512])
                nc.sync.dma_start(
                    out=xt[:, half:, :],
                    in_=xr[:, half:, st * 512:(st + 1) * 512])
                for t in range(QT_TILES):
                    pool = pspool or ps_big
                    ps = pool.tile([128, 512], F32, name="psqk",
                                   tag="psqp" if pspool else "psqk")
                    for c in range(KCH):
                        nc.tensor.matmul(
                            ps, w_chunks[:, c, t * 128:(t + 1) * 128],
                            xt[:, c, :], start=(c == 0), stop=(c == KCH - 1))
                    sl = slice(st * 512, (st + 1) * 512)
                    for half in range(2):
                        h = 2 * t + half
                        rows = slice(half * 64, half * 64 + 64)
                        nc.vector.tensor_scalar_add(
                            dst[h][0:64, sl], ps[rows, :], b_chunks[t][rows, :])
                        nc.vector.tensor_scalar_add(
                            dst[h][64:128, sl], ps[rows, :], b_chunks[t][rows, :])

        proj_dup(xk, wk_sb, bk_sb, kTd, "k")

        # --- V projection (fused bias + ones column via extra K row) ---
        for s in range(ST128):
            vt = vload.tile([128, KCH, 128], MM_DT, name="xv", tag="xv")
            nc.sync.dma_start(
                out=vt,
                in_=xv.ap().rearrange("(c p) s -> p c s", p=128)[
                    :, :, s * 128:(s + 1) * 128])
            ps = ps_big.tile([128, 512], F32, name="psv", tag="psqk")
            for c in range(KCH):
                nc.tensor.matmul(ps[:, :VROW], vt[:, c, :], wv_sb[:, c, :],
                                 start=(c == 0), stop=False)
            nc.tensor.matmul(ps[:, :VROW], ones1, wv_row,
                             start=False, stop=True)
            nc.vector.tensor_copy(v_sb[s], ps[:, :VROW])

        proj_ctx.close()
        attn_sb = ctx.enter_context(tc.tile_pool(name="attn_sb", bufs=8))
        stage = ctx.enter_context(tc.tile_pool(name="stage", bufs=2))
        misc = ctx.enter_context(tc.tile_pool(name="misc", bufs=1))
        outcp = ctx.enter_context(tc.tile_pool(name="outcp", bufs=3))

        # --- attention + output projection, qt-outer; Q proj streams in ---
        inv_scale = 1.0 / (2.0 * np.sqrt(DK))  # dup-dk doubles the dot
        proj_dup(xq, wq_sb, bq_sb, qTd, "q", sts=[0], pspool=ps_qp)
        for qt in range(ST512):
            qsl = slice(qt * 512, (qt + 1) * 512)
            ctx_stage = [stage.tile([VW, 512], F32, name=f"cs{h}", tag=f"cs{h}")
                         for h in range(HPC)]
            dn4 = misc.tile([128, 512], F32, name="dn4", tag="dn4")
            for h in range(HPC):
                if h == 2 and qt + 1 < ST512:
                    proj_dup(xq, wq_sb, bq_sb, qTd, "q", sts=[qt + 1],
                             pspool=ps_qp)
                pcs = [ps_acc.tile([VW, 512], F32, name=f"pc{j}", tag=f"pc{j}")
                       for j in range(2)]
                for kc in range(ST128):
                    ps = ps_big.tile([128, 512], F32, name="pss", tag="psqk")
                    nc.tensor.matmul(ps, kTd[h][:, kc * 128:(kc + 1) * 128],
                                     qTd[h][:, qsl], start=True, stop=True)
                    at = attn_sb.tile([128, 512], MM_DT, name="at", tag="at")
                    nc.scalar.activation(out=at, in_=ps, func=EXPF,
                                         scale=inv_scale)
                    nc.tensor.matmul(pcs[kc % 2],
                                     v_sb[kc][:, h * VW:(h + 1) * VW], at,
                                     start=(kc < 2), stop=(kc >= ST128 - 2))
                # combine the two banks; row 64 is the denominator
                tmp = misc.tile([VW, 512], F32, name="tmp", tag="tmp", bufs=2)
                nc.vector.tensor_copy(tmp, pcs[0])
                nc.vector.tensor_add(ctx_stage[h], pcs[1], tmp)
                nc.vector.tensor_copy(dn4[32 * h:32 * h + 1, :],
                                      ctx_stage[h][64:65, :])
            rec4 = misc.tile([128, 512], F32, name="rec4", tag="rec4")
            rscr = misc.tile([128, 512], F32, name="rscr", tag="rscr")
            nc.vector.reciprocal_approx_accurate(rec4[0:97, :], dn4[0:97, :],
                                                 rscr[0:97, :])
            cT = [stage.tile([128, 512], MM_DT, name=f"cT{t}", tag=f"cT{t}")
                  for t in range(QT_TILES)]
            for h in range(HPC):
                rec1 = misc.tile([1, 512], F32, name="rec1", tag="rec1")
                nc.vector.tensor_copy(rec1, rec4[32 * h:32 * h + 1, :])
                rb = misc.tile([64, 512], F32, name="rb", tag="rb")
                nc.gpsimd.partition_broadcast(rb, rec1)
                nc.vector.tensor_mul(
                    cT[h // 2][(h % 2) * 64:(h % 2) * 64 + 64, :],
                    ctx_stage[h][0:64, :], rb)
            # partial output projection for this qt
            for ot in range(D // 128):
                po = ps_out.tile([128, 512], F32, name="pso", tag="pso")
                for t in range(QT_TILES):
                    nc.tensor.matmul(po, wo_sb[t][:, ot * 128:(ot + 1) * 128],
                                     cT[t], start=(t == 0),
                                     stop=(t == QT_TILES - 1))
                yo = outcp.tile([128, 512], F32, name="yo", tag="yo")
                nc.vector.tensor_copy(yo, po)
                nc.sync.dma_start(out=yt[ot * 128:(ot + 1) * 128, qsl], in_=yo)

    nc.compile()
    return nc


_NC_CACHE = {}


def kernel(q, k, v, mask, Wq, bq, Wk, bk, Wv, bv, Wo, bo):
    global LAST_RESULTS
    q = np.asarray(q, dtype=np.float32)
    k = np.asarray(k, dtype=np.float32)
    v = np.asarray(v, dtype=np.float32)

    if "nc" not in _NC_CACHE:
        _NC_CACHE["nc"] = _build_nc()
    nc = _NC_CACHE["nc"]

    def cvt(a):
        return np.ascontiguousarray(np.asarray(a, dtype=np.float32)).astype(MM_NP)

    xqT = [cvt(q[b].T) for b in range(B)]
    xkT = [cvt(k[b].T) for b in range(B)]
    xvT = [cvt(v[b].T) for b in range(B)]

    in_maps = []
    for core in range(N_CORES):
        b, g = divmod(core, TP)
        F = slice(g * HF, (g + 1) * HF)
        # wv_ext: [D+1, VROW]; per head block of 65 cols = [Wv.T | bias/1]
        wv_ext = np.zeros((D + 1, VROW), dtype=np.float32)
        for j in range(HPC):
            feat = slice(g * HF + j * DK, g * HF + (j + 1) * DK)
            wv_ext[:D, j * VW:j * VW + DK] = np.asarray(Wv)[feat, :].T
            wv_ext[D, j * VW:j * VW + DK] = np.asarray(bv)[feat]
            wv_ext[D, j * VW + DK] = 1.0
        in_maps.append({
            "xqT": xqT[b], "xkT": xkT[b], "xvT": xvT[b],
            "wqT": cvt(np.asarray(Wq)[F, :].T),
            "wkT": cvt(np.asarray(Wk)[F, :].T),
            "bq": np.ascontiguousarray(np.asarray(bq)[F].reshape(HF, 1)),
            "bk": np.ascontiguousarray(np.asarray(bk)[F].reshape(HF, 1)),
            "wv_ext": wv_ext.astype(MM_NP),
            "woT": cvt(np.asarray(Wo)[:, F].T),
            "ones1": np.ones((1, 128), dtype=MM_NP),
        })

    res = run_bass_kernel_spmd(nc, in_maps, list(range(N_CORES)))
    LAST_RESULTS = res

    out = np.zeros((B, S, D), dtype=np.float32)
    for b in range(B):
        acc = np.zeros((D, S), dtype=np.float32)
        for g in range(TP):
            acc += res.results[b * TP + g]["yT"]
        out[b] = acc.T + np.asarray(bo)[None, :]
    return out
